# revision 22
# baseline (speedup 1.0000x reference)
"""Trainium2 Bass kernel for multi-head attention with RoPE.

Problem: B=2, S=2048, H=2048, 16 heads, head_dim=128.
  q/k/v = hidden @ W{q,k,v} + b{q,k,v}  (per-head reshape)
  q, k = rope(q), rope(k)   (interleaved rotate-half)
  out = softmax(q k^T / sqrt(hd)) v
  final = out @ Wo + bo

Sharding over 8 cores: core c = 4*b + g handles batch b and head group g
(4 heads = 512 of the 2048 hidden columns). Each core computes a partial
out-projection over its 512 columns; the host sums the 4 partials per batch
and adds bo.

Key layout/engine choices (v2, bf16):
- All matmul operands are bfloat16 (PSUM accumulation stays fp32). Measured
  on this silicon: bf16 matmuls in accumulation groups issue at ~238 ns per
  [128x128]x[128x512] vs ~294 ns for fp32r — LDWEIGHTS against bf16
  stationary tiles pipelines better. The 2e-2 rel-err budget dwarfs the
  bf16 quantization noise (~5e-3 worst case).
- Q/K are produced transposed (head_dim on partitions) by the projection
  matmuls with host-de-interleaved weight columns, so RoPE's rotate-half is
  a PE permutation matmul plus two contiguous half-tile multiplies, and
  attention scores S^T = K^T-tiles x Q^T need no transposes anywhere.
- Score matmuls write PAIRS of k-tiles into one [128,1024] PSUM tile
  (2 banks) so each exp is a single ACT instruction over 1024 columns:
  ACT runs at (N+352)/1.2 ns, so halving the instruction count cuts the
  per-instruction 293 ns overhead; phase-2 ACT drops under the PE time.
- Softmax row sums come from a DVE accumulation of the exp tiles (bf16
  adds) followed by ONE ones-vector matmul per (q-chunk, head) — replacing
  the 16 per-k-tile PE row-sum matmuls of v1 (256 matmuls removed).
- Out-projection matmul groups are interleaved between the attention score
  groups of the next q-chunk so the PE never drains while ACT catches up.
- Softmax skips max-subtraction: scores are ~N(0,1), exp cannot overflow.
"""

import math
import os

import numpy as np

os.environ.setdefault("JAX_COMPILATION_CACHE_DIR", "/tmp/jax_bass_cache")

import concourse.bass as bass  # noqa: E402
import concourse.mybir as mybir  # noqa: E402
import concourse.tile as tile  # noqa: E402
from concourse import bacc, bass_utils  # noqa: E402

try:
    import jax

    jax.config.update("jax_compilation_cache_dir", "/tmp/jax_bass_cache")
except Exception:
    pass

B, S, H = 2, 2048, 2048
NH, HD = 16, 128
NCORES = 8
HG = 4          # heads per core
DC = HG * HD    # 512 hidden columns per core
BASE = 10000.0

F32 = mybir.dt.float32
BF16 = mybir.dt.bfloat16

NS = S // 512      # 4 s-chunks of 512
NKT = S // 128     # 16 s tiles of 128
NHT = H // 128     # 16 contraction tiles of 128
SCALE = 1.0 / math.sqrt(HD)


def _build_program():
    nc = bacc.Bacc("TRN2", target_bir_lowering=False, debug=False)

    # packed layouts (see kernel()): per-partition rows are DRAM-contiguous
    xTs = nc.dram_tensor("xTs", [128, NS * NHT * 512], BF16, kind="ExternalInput")
    wqp = nc.dram_tensor("wqp", [128, NHT * DC], BF16, kind="ExternalInput")
    wkp = nc.dram_tensor("wkp", [128, NHT * DC], BF16, kind="ExternalInput")
    wvp = nc.dram_tensor("wvp", [128, NHT * DC], BF16, kind="ExternalInput")
    bqk = nc.dram_tensor("bqk", [128, 2 * HG], F32, kind="ExternalInput")
    bvb = nc.dram_tensor("bvb", [128, DC], F32, kind="ExternalInput")
    cosT = nc.dram_tensor("cosT", [128, S], BF16, kind="ExternalInput")
    sinTs = nc.dram_tensor("sinTs", [128, S], BF16, kind="ExternalInput")
    swap = nc.dram_tensor("swap", [128, 128], BF16, kind="ExternalInput")
    ones = nc.dram_tensor("ones", [128, 1], BF16, kind="ExternalInput")
    wo = nc.dram_tensor("wo", [DC, H], BF16, kind="ExternalInput")
    out = nc.dram_tensor("out", [S, H], F32, kind="ExternalOutput")

    with tile.TileContext(nc) as tc:
        with tc.tile_pool(name="persist", bufs=1) as pp:
            qt = [pp.tile([128, S], BF16, tag=f"qt{h}", name=f"qt{h}") for h in range(HG)]
            kt = [pp.tile([128, S], BF16, tag=f"kt{h}", name=f"kt{h}") for h in range(HG)]
            vt = [pp.tile([128, DC], BF16, tag=f"vt{t}", name=f"vt{t}") for t in range(NKT)]
            bqk_sb = pp.tile([128, 2 * HG], F32, tag="bqk", name="bqk_sb")
            bvb_sb = pp.tile([128, DC], F32, tag="bvb", name="bvb_sb")
            swap_sb = pp.tile([128, 128], BF16, tag="swap", name="swap_sb")
            ones_sb = pp.tile([128, 1], BF16, tag="ones", name="ones_sb")

            # ---------------- phase 1: projections + rope ----------------
            with tc.tile_pool(name="ph1", bufs=1) as p1, \
                 tc.tile_pool(name="ph1ps", bufs=8, space="PSUM") as ps1:

                def load_w_slab(wdram, w, n, label, split=False):
                    # [128, 2048] slab = contraction blocks 4w..4w+3
                    t = p1.tile([128, 2048], BF16, tag="wslab", bufs=5,
                                name=f"w_{label}_{n}_{w}")
                    if split:
                        # eighth-granularity on the very first slab: the first
                        # matmuls start as soon as their block lands; finer
                        # cuts measured slower (the extra descriptors delay
                        # the rest of the slab)
                        for q8 in range(8):
                            nc.sync.dma_start(
                                t[:, 256 * q8:256 * (q8 + 1)],
                                wdram[:, 2048 * w + 256 * q8:
                                      2048 * w + 256 * (q8 + 1)])
                    else:
                        nc.sync.dma_start(t[:],
                                          wdram[:, 2048 * w:2048 * (w + 1)])
                    return t

                # PE warm-up during the initial DMA wait: ~120 tiny matmuls on
                # a memset scratch keep the HAM activity window busy so the
                # first real matmuls run at 2.4 GHz instead of 1.2 GHz.
                warm_sb = p1.tile([128, 16], BF16, tag="warmmm", bufs=1,
                                  name="warm_sb")
                nc.vector.memset(warm_sb[:], 0.0)
                ps_warm = ps1.tile([1, 16], F32, tag="pp", name="ps_warm")
                for i in range(120):
                    nc.tensor.matmul(ps_warm[:], warm_sb[:, 0:1],
                                     warm_sb[:], start=(i == 0), stop=(i == 119))

                def rope_evict(ps_acc, dst_slice, bcol, n, h, which,
                               cos_sb, sin_sb):
                    # bias add (per-partition) while evicting PSUM -> SBUF
                    q0 = p1.tile([128, 512], BF16, tag="q0", bufs=2,
                                 name=f"q0_{n}_{h}_{which}")
                    nc.scalar.activation(
                        q0[:], ps_acc[:],
                        mybir.ActivationFunctionType.Identity,
                        bias=bqk_sb[:, bcol:bcol + 1], scale=1.0)
                    # rope: dst = q0*cos + swap(q0)*sin_signed
                    ps_sw = ps1.tile([128, 512], F32, tag="pp", name=f"sw{n}{h}{which}")
                    nc.tensor.matmul(ps_sw[:], swap_sb[:], q0[:],
                                     start=True, stop=True)
                    t1 = p1.tile([128, 512], BF16, tag="t1", bufs=2,
                                 name=f"t1_{n}_{h}_{which}")
                    nc.vector.tensor_tensor(t1[:], ps_sw[:], sin_sb[:],
                                            op=mybir.AluOpType.mult)
                    t2 = p1.tile([128, 512], BF16, tag="t2", bufs=2,
                                 name=f"t2_{n}_{h}_{which}")
                    nc.vector.tensor_tensor(t2[:], q0[:], cos_sb[:],
                                            op=mybir.AluOpType.mult)
                    nc.vector.tensor_tensor(dst_slice, t2[:], t1[:],
                                            op=mybir.AluOpType.add)

                for n in range(NS):
                    # xT slabs for this chunk; rotating tags prefetch the next
                    # chunk while this one computes; SWDGE ring keeps their
                    # slot waits off the weight stream on sync
                    xslabs = []
                    for w in range(4):
                        xs = p1.tile([128, 2048], BF16, tag=f"xs{(4 * n + w) % 6}",
                                     bufs=1, name=f"xs_{n}_{w}")
                        base = 8192 * n + 2048 * w
                        if n == 0:
                            # eighth-granularity so the first matmuls start as
                            # soon as their block lands, not the whole slab
                            for q8 in range(8 if w == 0 else 4):
                                step = 256 if w == 0 else 512
                                nc.gpsimd.dma_start(
                                    xs[:, step * q8:step * (q8 + 1)],
                                    xTs[:, base + step * q8:base + step * (q8 + 1)])
                        else:
                            nc.gpsimd.dma_start(xs[:], xTs[:, base:base + 2048])
                        xslabs.append(xs)
                    if n == NS - 1:
                        # warm the gpsimd PartitionBroadcast ucode library off
                        # the critical path: the first real broadcast in
                        # phase 2 otherwise eats a ~10us LIBRARY_RELOAD stall
                        # that chains into the PE via the PSUM WAR on po.
                        # Emitted after this chunk's SWDGE dma_starts so the
                        # DGE library is not reloaded again afterwards.
                        warm = p1.tile([128, 8], F32, tag="warm", bufs=1,
                                       name="gpsimd_warm")
                        nc.gpsimd.partition_broadcast(warm[:], bqk_sb[0:1, :])

                    def xt_block(ht):
                        s = xslabs[ht // 4]
                        return s[:, 512 * (ht % 4):512 * (ht % 4 + 1)]

                    # Q pass then K pass: contraction-block outer so every
                    # streamed weight slab serves all 4 heads
                    qaccs = kaccs = None
                    for which, wdram in ((0, wqp), (1, wkp)):
                        accs = [ps1.tile([128, 512], F32, tag="pp",
                                         name=f"acc{n}{which}{h}") for h in range(HG)]
                        if which == 0:
                            qaccs = accs
                        else:
                            kaccs = accs
                        for w in range(4):
                            wt = load_w_slab(wdram, w, n, f"qk{which}",
                                             split=(n == 0 and which == 0 and w == 0))
                            for hti in range(4):
                                ht = 4 * w + hti
                                for h in range(HG):
                                    nc.tensor.matmul(
                                        accs[h][:],
                                        wt[:, 512 * hti + 128 * h:
                                           512 * hti + 128 * (h + 1)],
                                        xt_block(ht),
                                        start=(ht == 0), stop=(ht == NHT - 1))
                        if which == 0:
                            # constants and rope tables are first needed by the
                            # evictions below; keep them off the critical
                            # startup path behind the first weight slabs
                            if n == 0:
                                nc.sync.dma_start(bqk_sb[:], bqk[:])
                                nc.sync.dma_start(bvb_sb[:], bvb[:])
                                nc.sync.dma_start(swap_sb[:], swap[:])
                                nc.sync.dma_start(ones_sb[:], ones[:])
                            cos_sb = p1.tile([128, 512], BF16, tag="cos", bufs=2,
                                             name=f"cos_{n}")
                            sin_sb = p1.tile([128, 512], BF16, tag="sin", bufs=2,
                                             name=f"sin_{n}")
                            ns = slice(512 * n, 512 * (n + 1))
                            nc.sync.dma_start(cos_sb[:], cosT[:, ns])
                            nc.sync.dma_start(sin_sb[:], sinTs[:, ns])

                    # evict+rope Q while the V matmuls stream
                    for h in range(HG):
                        rope_evict(qaccs[h], qt[h][:, 512 * n:512 * (n + 1)],
                                   h, n, h, 0, cos_sb, sin_sb)

                    # V pass (natural [s, d] layout); the K rope evictions are
                    # interleaved between the weight-slab groups so the DVE
                    # work spreads across the pass instead of piling up at the
                    # chunk boundary where it would hold PSUM banks hostage
                    vaccs = [ps1.tile([128, DC], F32, tag="pp",
                                      name=f"vacc{n}{t}") for t in range(4)]
                    for w in range(4):
                        wt = load_w_slab(wvp, w, n, "v")
                        # the last chunk front-loads its K evictions before
                        # the V matmuls so the rope chain of head 3 is not
                        # queued ahead of the first phase-2 score matmuls
                        if n == NS - 1 and w == 0:
                            for kh in range(HG):
                                rope_evict(kaccs[kh],
                                           kt[kh][:, 512 * n:512 * (n + 1)],
                                           HG + kh, n, kh, 1, cos_sb, sin_sb)
                        for hti in range(4):
                            ht = 4 * w + hti
                            for t in range(4):
                                nc.tensor.matmul(
                                    vaccs[t][:],
                                    xt_block(ht)[:, 128 * t:128 * (t + 1)],
                                    wt[:, 512 * hti:512 * (hti + 1)],
                                    start=(ht == 0), stop=(ht == NHT - 1))
                        if n != NS - 1:
                            rope_evict(kaccs[w], kt[w][:, 512 * n:512 * (n + 1)],
                                       HG + w, n, w, 1, cos_sb, sin_sb)

                    for t in range(4):
                        nc.vector.tensor_tensor(vt[4 * n + t][:], vaccs[t][:],
                                                bvb_sb[:], op=mybir.AluOpType.add)

            # ---------- phase 2+3: attention + output projection ----------
            with tc.tile_pool(name="ph2", bufs=1) as p2, \
                 tc.tile_pool(name="ph2ps", bufs=1, space="PSUM") as ps2:
                wot = []
                for dc in range(HG):
                    wt = p2.tile([128, H], BF16, tag=f"wo{dc}", name=f"wo{dc}")
                    nc.sync.dma_start(wt[:], wo[128 * dc:128 * (dc + 1), :])
                    wot.append(wt)

                def norm_chain(po, esum, at_h, qc, h):
                    # rowsum: one ones-vector matmul on the DVE-accumulated
                    # esum. Emitted HERE (a few score groups into the next
                    # block) so the PE never stalls waiting for the DVE
                    # esum chain at a block boundary.
                    # ps_r shares the pf rotation (it is tiny and its
                    # producers/consumers are far from the outproj groups
                    # in time), freeing a PSUM bank so pf double-buffers
                    pr = ps2.tile([1, 512], F32, tag="pf", bufs=2,
                                  name=f"pr{qc}{h}")
                    nc.tensor.matmul(pr[:], ones_sb[:], esum[:],
                                     start=True, stop=True)
                    # at = po / rowsum: approx reciprocal (18 bits, plenty),
                    # gpsimd partition-broadcast, DVE mult
                    r_sb = p2.tile([1, 512], F32, tag="r_sb", bufs=2,
                                   name=f"rsb{qc}{h}")
                    nc.vector.tensor_copy(r_sb[:], pr[:])
                    recip = p2.tile([1, 512], F32, tag="recip", bufs=2,
                                    name=f"recip{qc}{h}")
                    nc.vector.reciprocal_approx_fast(recip[:], r_sb[:])
                    rb = p2.tile([128, 512], F32, tag="rb", bufs=2,
                                 name=f"rbs{qc}{h}")
                    nc.gpsimd.partition_broadcast(rb[:], recip[:])
                    nc.vector.tensor_tensor(at_h[:], po[:], rb[:],
                                            op=mybir.AluOpType.mult)

                def emit_outproj_group(qc, ats, gidx, tail=False):
                    # one quarter of the out-projection of q-chunk qc:
                    # rows 128*gidx.. of the chunk x all H columns in 4 groups
                    ti, hc = divmod(gidx, NS)
                    rs_out = slice(512 * qc + 128 * ti,
                                   512 * qc + 128 * (ti + 1))
                    cs = slice(512 * hc, 512 * (hc + 1))
                    # the tail (no attention work interleaved) rotates across
                    # the freed po banks too, so back-to-back groups never
                    # wait on a single bank's eviction
                    tag = "po" if (tail and gidx % 2 == 1) else "pf"
                    ps_f = ps2.tile([128, 512], F32, tag=tag, bufs=2,
                                    name=f"pf{qc}{ti}{hc}")
                    for dc in range(HG):
                        nc.tensor.matmul(
                            ps_f[:], ats[dc][:, 128 * ti:128 * (ti + 1)],
                            wot[dc][:, cs],
                            start=(dc == 0), stop=(dc == HG - 1))
                    ost = p2.tile([128, 512], F32, tag="ost", bufs=3,
                                  name=f"ost{qc}{ti}{hc}")
                    # alternate the PSUM eviction between DVE and ACT so
                    # neither engine becomes the phase-2 straggler
                    if tail:
                        # tail: both engines evict half each, concurrently,
                        # and halves ride separate DMA queues for a short
                        # drain after the last matmul
                        nc.vector.tensor_copy(ost[:, 0:256], ps_f[:, 0:256])
                        nc.scalar.activation(
                            ost[:, 256:512], ps_f[:, 256:512],
                            mybir.ActivationFunctionType.Identity, scale=1.0)
                        nc.sync.dma_start(out[rs_out, 512 * hc:512 * hc + 256],
                                          ost[:, 0:256])
                        nc.sync.dma_start(out[rs_out, 512 * hc + 256:512 * (hc + 1)],
                                          ost[:, 256:512])
                    else:
                        if gidx % 2 == 0:
                            nc.vector.tensor_copy(ost[:], ps_f[:])
                        else:
                            nc.scalar.activation(
                                ost[:], ps_f[:],
                                mybir.ActivationFunctionType.Identity, scale=1.0)
                        nc.sync.dma_start(out[rs_out, cs], ost[:])

                pending = None       # (po, pr, at, qc, h) awaiting normalization
                pending_ats = None   # previous chunk's at tiles awaiting out-proj
                for qc in range(NS):
                    qs = slice(512 * qc, 512 * (qc + 1))
                    ats = []
                    for h in range(HG):
                        hs = slice(128 * h, 128 * (h + 1))
                        ps_o = ps2.tile([128, 512], F32, tag="po", bufs=2,
                                        name=f"po{qc}{h}")
                        esum = p2.tile([128, 512], BF16, tag="esum", bufs=2,
                                       name=f"esum{qc}{h}")
                        # 8 score pair-groups; each becomes one [128,1024] exp
                        exps = []

                        def emit_scores(g, qc=qc, h=h, qs=qs):
                            ps_s = ps2.tile([128, 1024], F32, tag="ps", bufs=2,
                                            name=f"pss{qc}{h}{g}")
                            for j in range(2):
                                ks = slice(128 * (2 * g + j), 128 * (2 * g + j + 1))
                                nc.tensor.matmul(ps_s[:, 512 * j:512 * (j + 1)],
                                                 kt[h][:, ks], qt[h][:, qs],
                                                 start=True, stop=True)
                            e = p2.tile([128, 1024], BF16, tag="e", bufs=4,
                                        name=f"e{qc}{h}{g}")
                            nc.scalar.activation(e[:], ps_s[:],
                                                 mybir.ActivationFunctionType.Exp,
                                                 scale=SCALE)
                            exps.append(e)

                        emit_scores(0)
                        for g in range(8):
                            if g + 1 < 8:
                                emit_scores(g + 1)
                            # deferred norm of the previous (qc, h) block:
                            # emitted after two score groups so its rowsum
                            # matmul has PE runway while the previous esum
                            # finishes on DVE
                            if g == 0 and pending is not None:
                                norm_chain(*pending)
                                pending = None
                            e = exps[g]
                            for j in range(2):
                                kti = 2 * g + j
                                nc.tensor.matmul(
                                    ps_o[:], vt[kti][:, hs],
                                    e[:, 512 * j:512 * (j + 1)],
                                    start=(kti == 0), stop=(kti == NKT - 1))
                            # rowsum accumulation on DVE (bf16):
                            # ep = e_lo + e_hi ; esum += ep
                            ep = p2.tile([128, 512], BF16, tag="ep", bufs=2,
                                         name=f"ep{qc}{h}{g}")
                            nc.vector.tensor_tensor(ep[:], e[:, 0:512],
                                                    e[:, 512:1024],
                                                    op=mybir.AluOpType.add)
                            if g == 0:
                                esum_prev = ep
                            else:
                                dst = esum if g == 7 else \
                                    p2.tile([128, 512], BF16, tag="esa", bufs=2,
                                            name=f"esa{qc}{h}{g}")
                                nc.vector.tensor_tensor(dst[:], esum_prev[:],
                                                        ep[:],
                                                        op=mybir.AluOpType.add)
                                esum_prev = dst
                            # interleave one out-proj group of the previous
                            # chunk behind every other score/AV group
                            if g % 2 == 1 and pending_ats is not None:
                                emit_outproj_group(qc - 1, pending_ats,
                                                  4 * h + g // 2)

                        at_h = p2.tile([128, 512], BF16, tag=f"at{h}", bufs=2,
                                       name=f"at{qc}{h}")
                        ats.append(at_h)
                        pending = (ps_o, esum, at_h, qc, h)
                    pending_ats = ats
                norm_chain(*pending)
                for gidx in range(16):
                    emit_outproj_group(NS - 1, pending_ats, gidx, tail=True)

    nc.finalize()
    return nc


_NC = None
LAST_RESULTS = None


def _rope_tables():
    j = np.arange(HD // 2, dtype=np.float64)
    inv_freq = BASE ** (-2.0 * j / HD)
    pos = np.arange(S, dtype=np.float64)
    ang = pos[None, :] * inv_freq[:, None]          # [64, S]
    cos = np.cos(ang)
    sin = np.sin(ang)
    cosT = np.concatenate([cos, cos], axis=0)       # [128, S]
    sinTs = np.concatenate([-sin, sin], axis=0)     # [128, S]
    return cosT, sinTs


def _pack_w(w):
    # [H, DC] -> [128, NHT*DC]: block j holds rows 128j..128j+127; each
    # SBUF partition's slab row is contiguous in DRAM
    return np.ascontiguousarray(
        w.reshape(NHT, 128, DC).transpose(1, 0, 2).reshape(128, NHT * DC))


def kernel(hidden_state, Wq, bq, Wk, bk, Wv, bv, Wo, bo):
    global _NC, LAST_RESULTS
    import ml_dtypes  # noqa: F401

    bf = np.dtype("bfloat16")
    hidden_state = np.asarray(hidden_state, dtype=np.float32)
    Wq, bq = np.asarray(Wq, np.float32), np.asarray(bq, np.float32)
    Wk, bk = np.asarray(Wk, np.float32), np.asarray(bk, np.float32)
    Wv, bv = np.asarray(Wv, np.float32), np.asarray(bv, np.float32)
    Wo, bo = np.asarray(Wo, np.float32), np.asarray(bo, np.float32)

    if _NC is None:
        _NC = _build_program()

    cosT, sinTs = _rope_tables()
    cosT = np.ascontiguousarray(cosT.astype(bf))
    sinTs = np.ascontiguousarray(sinTs.astype(bf))
    perm = np.concatenate([np.arange(0, HD, 2), np.arange(1, HD, 2)])
    swap_m = np.zeros((128, 128), np.float32)
    for m in range(128):
        swap_m[(m + 64) % 128, m] = 1.0
    swap_m = swap_m.astype(bf)
    ones_v = np.ones((128, 1), bf)

    in_maps = []
    for c in range(NCORES):
        b, g = divmod(c, HG)
        cols = np.arange(DC) + DC * g
        # per-head de-interleave permutation for Q/K columns
        pcols = np.concatenate([DC * g + HD * h + perm for h in range(HG)])
        bqk_np = np.concatenate([bq[pcols].reshape(HG, HD).T,
                                 bk[pcols].reshape(HG, HD).T], axis=1)
        # hidden^T packed: [s-chunk n][block j][s within chunk] contiguous
        # per partition: xTs[p, n, j, c] = hidden[b][512n+c, 128j+p]
        xT = hidden_state[b].T                       # [H, S]
        xTs = (xT.reshape(NHT, 128, NS, 512)
               .transpose(1, 2, 0, 3).reshape(128, NS * NHT * 512))
        in_maps.append({
            "xTs": np.ascontiguousarray(xTs.astype(bf)),
            "wqp": _pack_w(Wq[:, pcols].astype(bf)),
            "wkp": _pack_w(Wk[:, pcols].astype(bf)),
            "wvp": _pack_w(Wv[:, cols].astype(bf)),
            "bqk": np.ascontiguousarray(bqk_np),
            "bvb": np.ascontiguousarray(np.tile(bv[cols], (128, 1))),
            "cosT": cosT,
            "sinTs": sinTs,
            "swap": swap_m,
            "ones": ones_v,
            "wo": np.ascontiguousarray(Wo[cols, :].astype(bf)),
        })

    trace = bool(os.environ.get("KERNEL_TRACE"))
    res = bass_utils.run_bass_kernel_spmd(_NC, in_maps, core_ids=list(range(NCORES)),
                                          trace=trace)
    LAST_RESULTS = res

    out = np.zeros((B, S, H), np.float32)
    for c in range(NCORES):
        b = c // HG
        out[b] += res.results[c]["out"]
    out += bo[None, None, :]
    return out


# revision 28
# speedup vs baseline: 1.0106x; 1.0106x over previous
"""Trainium2 Bass kernel for multi-head attention with RoPE.

Problem: B=2, S=2048, H=2048, 16 heads, head_dim=128.
  q/k/v = hidden @ W{q,k,v} + b{q,k,v}  (per-head reshape)
  q, k = rope(q), rope(k)   (interleaved rotate-half)
  out = softmax(q k^T / sqrt(hd)) v
  final = out @ Wo + bo

Sharding over 8 cores: core c = 4*b + g handles batch b and head group g
(4 heads = 512 of the 2048 hidden columns). Each core computes a partial
out-projection over its 512 columns; the host sums the 4 partials per batch
and adds bo.

Key layout/engine choices (v2, bf16):
- All matmul operands are bfloat16 (PSUM accumulation stays fp32). Measured
  on this silicon: bf16 matmuls in accumulation groups issue at ~238 ns per
  [128x128]x[128x512] vs ~294 ns for fp32r — LDWEIGHTS against bf16
  stationary tiles pipelines better. The 2e-2 rel-err budget dwarfs the
  bf16 quantization noise (~5e-3 worst case).
- Q/K are produced transposed (head_dim on partitions) by the projection
  matmuls with host-de-interleaved weight columns, so RoPE's rotate-half is
  a PE permutation matmul plus two contiguous half-tile multiplies, and
  attention scores S^T = K^T-tiles x Q^T need no transposes anywhere.
- Score matmuls write PAIRS of k-tiles into one [128,1024] PSUM tile
  (2 banks) so each exp is a single ACT instruction over 1024 columns:
  ACT runs at (N+352)/1.2 ns, so halving the instruction count cuts the
  per-instruction 293 ns overhead; phase-2 ACT drops under the PE time.
- Softmax row sums come from a DVE accumulation of the exp tiles (bf16
  adds) followed by ONE ones-vector matmul per (q-chunk, head) — replacing
  the 16 per-k-tile PE row-sum matmuls of v1 (256 matmuls removed).
- Out-projection matmul groups are interleaved between the attention score
  groups of the next q-chunk so the PE never drains while ACT catches up.
- Softmax skips max-subtraction: scores are ~N(0,1), exp cannot overflow.
"""

import math
import os

import numpy as np

os.environ.setdefault("JAX_COMPILATION_CACHE_DIR", "/tmp/jax_bass_cache")

import concourse.bass as bass  # noqa: E402
import concourse.mybir as mybir  # noqa: E402
import concourse.tile as tile  # noqa: E402
from concourse import bacc, bass_utils  # noqa: E402

try:
    import jax

    jax.config.update("jax_compilation_cache_dir", "/tmp/jax_bass_cache")
except Exception:
    pass

B, S, H = 2, 2048, 2048
NH, HD = 16, 128
NCORES = 8
HG = 4          # heads per core
DC = HG * HD    # 512 hidden columns per core
BASE = 10000.0

F32 = mybir.dt.float32
BF16 = mybir.dt.bfloat16

NS = S // 512      # 4 s-chunks of 512
NKT = S // 128     # 16 s tiles of 128
NHT = H // 128     # 16 contraction tiles of 128
SCALE = 1.0 / math.sqrt(HD)


def _build_program():
    nc = bacc.Bacc("TRN2", target_bir_lowering=False, debug=False)

    # packed layouts (see kernel()): per-partition rows are DRAM-contiguous
    xTs = nc.dram_tensor("xTs", [128, NS * NHT * 512], BF16, kind="ExternalInput")
    wqp = nc.dram_tensor("wqp", [128, NHT * DC], BF16, kind="ExternalInput")
    wkp = nc.dram_tensor("wkp", [128, NHT * DC], BF16, kind="ExternalInput")
    wvp = nc.dram_tensor("wvp", [128, NHT * DC], BF16, kind="ExternalInput")
    bqk = nc.dram_tensor("bqk", [128, 2 * HG], F32, kind="ExternalInput")
    bvb = nc.dram_tensor("bvb", [128, DC], F32, kind="ExternalInput")
    cosT = nc.dram_tensor("cosT", [128, S], BF16, kind="ExternalInput")
    sinTs = nc.dram_tensor("sinTs", [128, S], BF16, kind="ExternalInput")
    swap = nc.dram_tensor("swap", [128, 128], BF16, kind="ExternalInput")
    ones = nc.dram_tensor("ones", [128, 1], BF16, kind="ExternalInput")
    wo = nc.dram_tensor("wo", [DC, H], BF16, kind="ExternalInput")
    out = nc.dram_tensor("out", [S, H], F32, kind="ExternalOutput")

    with tile.TileContext(nc) as tc:
        with tc.tile_pool(name="persist", bufs=1) as pp:
            qt = [pp.tile([128, S], BF16, tag=f"qt{h}", name=f"qt{h}") for h in range(HG)]
            kt = [pp.tile([128, S], BF16, tag=f"kt{h}", name=f"kt{h}") for h in range(HG)]
            vt = [pp.tile([128, DC], BF16, tag=f"vt{t}", name=f"vt{t}") for t in range(NKT)]
            bqk_sb = pp.tile([128, 2 * HG], F32, tag="bqk", name="bqk_sb")
            bvb_sb = pp.tile([128, DC], F32, tag="bvb", name="bvb_sb")
            swap_sb = pp.tile([128, 128], BF16, tag="swap", name="swap_sb")
            ones_sb = pp.tile([128, 1], BF16, tag="ones", name="ones_sb")

            # ---------------- phase 1: projections + rope ----------------
            with tc.tile_pool(name="ph1", bufs=1) as p1, \
                 tc.tile_pool(name="ph1ps", bufs=8, space="PSUM") as ps1:

                def load_w_slab(wdram, w, n, label, split=False):
                    # [128, 2048] slab = contraction blocks 4w..4w+3
                    t = p1.tile([128, 2048], BF16, tag="wslab", bufs=5,
                                name=f"w_{label}_{n}_{w}")
                    if split:
                        # eighth-granularity on the very first slab: the first
                        # matmuls start as soon as their block lands; finer
                        # cuts measured slower (the extra descriptors delay
                        # the rest of the slab)
                        for q8 in range(8):
                            nc.sync.dma_start(
                                t[:, 256 * q8:256 * (q8 + 1)],
                                wdram[:, 2048 * w + 256 * q8:
                                      2048 * w + 256 * (q8 + 1)])
                    else:
                        nc.sync.dma_start(t[:],
                                          wdram[:, 2048 * w:2048 * (w + 1)])
                    return t

                # PE warm-up during the initial DMA wait: ~120 tiny matmuls on
                # a memset scratch keep the HAM activity window busy so the
                # first real matmuls run at 2.4 GHz instead of 1.2 GHz.
                warm_sb = p1.tile([128, 16], BF16, tag="warmmm", bufs=1,
                                  name="warm_sb")
                nc.vector.memset(warm_sb[:], 0.0)
                ps_warm = ps1.tile([1, 16], F32, tag="pp", name="ps_warm")
                for i in range(120):
                    nc.tensor.matmul(ps_warm[:], warm_sb[:, 0:1],
                                     warm_sb[:], start=(i == 0), stop=(i == 119))

                def rope_evict(ps_acc, dst_slice, bcol, n, h, which,
                               cos_sb, sin_sb):
                    # bias add (per-partition) while evicting PSUM -> SBUF
                    q0 = p1.tile([128, 512], BF16, tag="q0", bufs=2,
                                 name=f"q0_{n}_{h}_{which}")
                    nc.scalar.activation(
                        q0[:], ps_acc[:],
                        mybir.ActivationFunctionType.Identity,
                        bias=bqk_sb[:, bcol:bcol + 1], scale=1.0)
                    # rope: dst = q0*cos + swap(q0)*sin_signed
                    ps_sw = ps1.tile([128, 512], F32, tag="pp", name=f"sw{n}{h}{which}")
                    nc.tensor.matmul(ps_sw[:], swap_sb[:], q0[:],
                                     start=True, stop=True)
                    t1 = p1.tile([128, 512], BF16, tag="t1", bufs=2,
                                 name=f"t1_{n}_{h}_{which}")
                    nc.vector.tensor_tensor(t1[:], ps_sw[:], sin_sb[:],
                                            op=mybir.AluOpType.mult)
                    t2 = p1.tile([128, 512], BF16, tag="t2", bufs=2,
                                 name=f"t2_{n}_{h}_{which}")
                    nc.vector.tensor_tensor(t2[:], q0[:], cos_sb[:],
                                            op=mybir.AluOpType.mult)
                    nc.vector.tensor_tensor(dst_slice, t2[:], t1[:],
                                            op=mybir.AluOpType.add)

                for n in range(NS):
                    # xT slabs for this chunk; rotating tags prefetch the next
                    # chunk while this one computes; SWDGE ring keeps their
                    # slot waits off the weight stream on sync
                    xslabs = []
                    for w in range(4):
                        xs = p1.tile([128, 2048], BF16, tag=f"xs{(4 * n + w) % 6}",
                                     bufs=1, name=f"xs_{n}_{w}")
                        base = 8192 * n + 2048 * w
                        if n == 0:
                            # eighth-granularity so the first matmuls start as
                            # soon as their block lands, not the whole slab
                            for q8 in range(8 if w == 0 else 4):
                                step = 256 if w == 0 else 512
                                nc.gpsimd.dma_start(
                                    xs[:, step * q8:step * (q8 + 1)],
                                    xTs[:, base + step * q8:base + step * (q8 + 1)])
                        else:
                            nc.gpsimd.dma_start(xs[:], xTs[:, base:base + 2048])
                        xslabs.append(xs)
                    if n == NS - 1:
                        # warm the gpsimd PartitionBroadcast ucode library off
                        # the critical path: the first real broadcast in
                        # phase 2 otherwise eats a ~10us LIBRARY_RELOAD stall
                        # that chains into the PE via the PSUM WAR on po.
                        # Emitted after this chunk's SWDGE dma_starts so the
                        # DGE library is not reloaded again afterwards.
                        warm = p1.tile([128, 8], F32, tag="warm", bufs=1,
                                       name="gpsimd_warm")
                        nc.gpsimd.partition_broadcast(warm[:], bqk_sb[0:1, :])

                    def xt_block(ht):
                        s = xslabs[ht // 4]
                        return s[:, 512 * (ht % 4):512 * (ht % 4 + 1)]

                    # Q pass then K pass: contraction-block outer so every
                    # streamed weight slab serves all 4 heads
                    qaccs = kaccs = None
                    for which, wdram in ((0, wqp), (1, wkp)):
                        accs = [ps1.tile([128, 512], F32, tag="pp",
                                         name=f"acc{n}{which}{h}") for h in range(HG)]
                        if which == 0:
                            qaccs = accs
                        else:
                            kaccs = accs
                        for w in range(4):
                            wt = load_w_slab(wdram, w, n, f"qk{which}",
                                             split=(n == 0 and which == 0 and w == 0))
                            for hti in range(4):
                                ht = 4 * w + hti
                                for h in range(HG):
                                    nc.tensor.matmul(
                                        accs[h][:],
                                        wt[:, 512 * hti + 128 * h:
                                           512 * hti + 128 * (h + 1)],
                                        xt_block(ht),
                                        start=(ht == 0), stop=(ht == NHT - 1))
                        if which == 0:
                            # constants and rope tables are first needed by the
                            # evictions below; keep them off the critical
                            # startup path behind the first weight slabs
                            if n == 0:
                                nc.sync.dma_start(bqk_sb[:], bqk[:])
                                nc.sync.dma_start(bvb_sb[:], bvb[:])
                                nc.sync.dma_start(swap_sb[:], swap[:])
                                nc.sync.dma_start(ones_sb[:], ones[:])
                            cos_sb = p1.tile([128, 512], BF16, tag="cos", bufs=2,
                                             name=f"cos_{n}")
                            sin_sb = p1.tile([128, 512], BF16, tag="sin", bufs=2,
                                             name=f"sin_{n}")
                            ns = slice(512 * n, 512 * (n + 1))
                            nc.sync.dma_start(cos_sb[:], cosT[:, ns])
                            nc.sync.dma_start(sin_sb[:], sinTs[:, ns])

                    # evict+rope Q while the V matmuls stream
                    for h in range(HG):
                        rope_evict(qaccs[h], qt[h][:, 512 * n:512 * (n + 1)],
                                   h, n, h, 0, cos_sb, sin_sb)

                    # V pass (natural [s, d] layout); the K rope evictions are
                    # interleaved between the weight-slab groups so the DVE
                    # work spreads across the pass instead of piling up at the
                    # chunk boundary where it would hold PSUM banks hostage
                    vaccs = [ps1.tile([128, DC], F32, tag="pp",
                                      name=f"vacc{n}{t}") for t in range(4)]
                    for w in range(4):
                        wt = load_w_slab(wvp, w, n, "v")
                        for hti in range(4):
                            ht = 4 * w + hti
                            for t in range(4):
                                nc.tensor.matmul(
                                    vaccs[t][:],
                                    xt_block(ht)[:, 128 * t:128 * (t + 1)],
                                    wt[:, 512 * hti:512 * (hti + 1)],
                                    start=(ht == 0), stop=(ht == NHT - 1))
                        rope_evict(kaccs[w], kt[w][:, 512 * n:512 * (n + 1)],
                                   HG + w, n, w, 1, cos_sb, sin_sb)

                    for t in range(4):
                        nc.vector.tensor_tensor(vt[4 * n + t][:], vaccs[t][:],
                                                bvb_sb[:], op=mybir.AluOpType.add)

            # ---------- phase 2+3: attention + output projection ----------
            with tc.tile_pool(name="ph2", bufs=1) as p2, \
                 tc.tile_pool(name="ph2ps", bufs=1, space="PSUM") as ps2:
                wot = []
                for dc in range(HG):
                    wt = p2.tile([128, H], BF16, tag=f"wo{dc}", name=f"wo{dc}")
                    nc.sync.dma_start(wt[:], wo[128 * dc:128 * (dc + 1), :])
                    wot.append(wt)

                def norm_chain(po, pr, at_h, qc, h):
                    # at = po / rowsum: approx reciprocal (18 bits, plenty),
                    # gpsimd partition-broadcast, DVE mult
                    r_sb = p2.tile([1, 512], F32, tag="r_sb", bufs=2,
                                   name=f"rsb{qc}{h}")
                    nc.vector.tensor_copy(r_sb[:], pr[:])
                    recip = p2.tile([1, 512], F32, tag="recip", bufs=2,
                                    name=f"recip{qc}{h}")
                    nc.vector.reciprocal_approx_fast(recip[:], r_sb[:])
                    rb = p2.tile([128, 512], F32, tag="rb", bufs=2,
                                 name=f"rbs{qc}{h}")
                    nc.gpsimd.partition_broadcast(rb[:], recip[:])
                    nc.vector.tensor_tensor(at_h[:], po[:], rb[:],
                                            op=mybir.AluOpType.mult)

                def emit_outproj_group(qc, ats, gidx, tail=False):
                    # one quarter of the out-projection of q-chunk qc:
                    # rows 128*gidx.. of the chunk x all H columns in 4 groups
                    ti, hc = divmod(gidx, NS)
                    rs_out = slice(512 * qc + 128 * ti,
                                   512 * qc + 128 * (ti + 1))
                    cs = slice(512 * hc, 512 * (hc + 1))
                    # the tail (no attention work interleaved) rotates across
                    # the freed po banks too, so back-to-back groups never
                    # wait on a single bank's eviction
                    tag = "po" if (tail and gidx % 2 == 1) else "pf"
                    ps_f = ps2.tile([128, 512], F32, tag=tag, bufs=2,
                                    name=f"pf{qc}{ti}{hc}")
                    for dc in range(HG):
                        nc.tensor.matmul(
                            ps_f[:], ats[dc][:, 128 * ti:128 * (ti + 1)],
                            wot[dc][:, cs],
                            start=(dc == 0), stop=(dc == HG - 1))
                    ost = p2.tile([128, 512], F32, tag="ost", bufs=3,
                                  name=f"ost{qc}{ti}{hc}")
                    # alternate the PSUM eviction between DVE and ACT so
                    # neither engine becomes the phase-2 straggler
                    if gidx % 2 == 0:
                        nc.vector.tensor_copy(ost[:], ps_f[:])
                    else:
                        nc.scalar.activation(
                            ost[:], ps_f[:],
                            mybir.ActivationFunctionType.Identity, scale=1.0)
                    if tail:
                        # halves ride separate DMA queues: shorter drain after
                        # the last matmul
                        nc.sync.dma_start(out[rs_out, 512 * hc:512 * hc + 256],
                                          ost[:, 0:256])
                        nc.sync.dma_start(out[rs_out, 512 * hc + 256:512 * (hc + 1)],
                                          ost[:, 256:512])
                    else:
                        nc.sync.dma_start(out[rs_out, cs], ost[:])

                pending = None       # (po, pr, at, qc, h) awaiting normalization
                pending_ats = None   # previous chunk's at tiles awaiting out-proj
                for qc in range(NS):
                    qs = slice(512 * qc, 512 * (qc + 1))
                    ats = []
                    for h in range(HG):
                        hs = slice(128 * h, 128 * (h + 1))
                        ps_o = ps2.tile([128, 512], F32, tag="po", bufs=2,
                                        name=f"po{qc}{h}")
                        # deferred norm of the previous (qc, h) block rides
                        # behind this block's matmul stream
                        if pending is not None:
                            norm_chain(*pending)
                            pending = None
                        esum = p2.tile([128, 512], BF16, tag="esum", bufs=2,
                                       name=f"esum{qc}{h}")
                        # 8 score pair-groups; each becomes one [128,1024] exp
                        exps = []

                        def emit_scores(g, qc=qc, h=h, qs=qs):
                            ps_s = ps2.tile([128, 1024], F32, tag="ps", bufs=2,
                                            name=f"pss{qc}{h}{g}")
                            for j in range(2):
                                ks = slice(128 * (2 * g + j), 128 * (2 * g + j + 1))
                                nc.tensor.matmul(ps_s[:, 512 * j:512 * (j + 1)],
                                                 kt[h][:, ks], qt[h][:, qs],
                                                 start=True, stop=True)
                            e = p2.tile([128, 1024], BF16, tag="e", bufs=4,
                                        name=f"e{qc}{h}{g}")
                            nc.scalar.activation(e[:], ps_s[:],
                                                 mybir.ActivationFunctionType.Exp,
                                                 scale=SCALE)
                            exps.append(e)

                        emit_scores(0)
                        for g in range(8):
                            if g + 1 < 8:
                                emit_scores(g + 1)
                            e = exps[g]
                            for j in range(2):
                                kti = 2 * g + j
                                nc.tensor.matmul(
                                    ps_o[:], vt[kti][:, hs],
                                    e[:, 512 * j:512 * (j + 1)],
                                    start=(kti == 0), stop=(kti == NKT - 1))
                            # rowsum accumulation on DVE (bf16):
                            # ep = e_lo + e_hi ; esum += ep
                            ep = p2.tile([128, 512], BF16, tag="ep", bufs=2,
                                         name=f"ep{qc}{h}{g}")
                            nc.vector.tensor_tensor(ep[:], e[:, 0:512],
                                                    e[:, 512:1024],
                                                    op=mybir.AluOpType.add)
                            if g == 0:
                                esum_prev = ep
                            else:
                                dst = esum if g == 7 else \
                                    p2.tile([128, 512], BF16, tag="esa", bufs=2,
                                            name=f"esa{qc}{h}{g}")
                                nc.vector.tensor_tensor(dst[:], esum_prev[:],
                                                        ep[:],
                                                        op=mybir.AluOpType.add)
                                esum_prev = dst
                            # interleave one out-proj group of the previous
                            # chunk behind every other score/AV group
                            if g % 2 == 1 and pending_ats is not None:
                                emit_outproj_group(qc - 1, pending_ats,
                                                  4 * h + g // 2)

                        # ps_r shares the pf rotation (it is tiny and its
                        # producers/consumers are far from the outproj groups
                        # in time), freeing a PSUM bank so pf double-buffers
                        ps_r = ps2.tile([1, 512], F32, tag="pf", bufs=2,
                                        name=f"pr{qc}{h}")
                        nc.tensor.matmul(ps_r[:], ones_sb[:], esum[:],
                                         start=True, stop=True)
                        at_h = p2.tile([128, 512], BF16, tag=f"at{h}", bufs=2,
                                       name=f"at{qc}{h}")
                        ats.append(at_h)
                        pending = (ps_o, ps_r, at_h, qc, h)
                    pending_ats = ats
                norm_chain(*pending)
                for gidx in range(16):
                    emit_outproj_group(NS - 1, pending_ats, gidx, tail=True)

    nc.finalize()
    return nc


_NC = None
LAST_RESULTS = None


def _rope_tables():
    j = np.arange(HD // 2, dtype=np.float64)
    inv_freq = BASE ** (-2.0 * j / HD)
    pos = np.arange(S, dtype=np.float64)
    ang = pos[None, :] * inv_freq[:, None]          # [64, S]
    cos = np.cos(ang)
    sin = np.sin(ang)
    cosT = np.concatenate([cos, cos], axis=0)       # [128, S]
    sinTs = np.concatenate([-sin, sin], axis=0)     # [128, S]
    return cosT, sinTs


def _pack_w(w):
    # [H, DC] -> [128, NHT*DC]: block j holds rows 128j..128j+127; each
    # SBUF partition's slab row is contiguous in DRAM
    return np.ascontiguousarray(
        w.reshape(NHT, 128, DC).transpose(1, 0, 2).reshape(128, NHT * DC))


def kernel(hidden_state, Wq, bq, Wk, bk, Wv, bv, Wo, bo):
    global _NC, LAST_RESULTS
    import ml_dtypes  # noqa: F401

    bf = np.dtype("bfloat16")
    hidden_state = np.asarray(hidden_state, dtype=np.float32)
    Wq, bq = np.asarray(Wq, np.float32), np.asarray(bq, np.float32)
    Wk, bk = np.asarray(Wk, np.float32), np.asarray(bk, np.float32)
    Wv, bv = np.asarray(Wv, np.float32), np.asarray(bv, np.float32)
    Wo, bo = np.asarray(Wo, np.float32), np.asarray(bo, np.float32)

    if _NC is None:
        _NC = _build_program()

    cosT, sinTs = _rope_tables()
    cosT = np.ascontiguousarray(cosT.astype(bf))
    sinTs = np.ascontiguousarray(sinTs.astype(bf))
    perm = np.concatenate([np.arange(0, HD, 2), np.arange(1, HD, 2)])
    swap_m = np.zeros((128, 128), np.float32)
    for m in range(128):
        swap_m[(m + 64) % 128, m] = 1.0
    swap_m = swap_m.astype(bf)
    ones_v = np.ones((128, 1), bf)

    in_maps = []
    for c in range(NCORES):
        b, g = divmod(c, HG)
        cols = np.arange(DC) + DC * g
        # per-head de-interleave permutation for Q/K columns
        pcols = np.concatenate([DC * g + HD * h + perm for h in range(HG)])
        bqk_np = np.concatenate([bq[pcols].reshape(HG, HD).T,
                                 bk[pcols].reshape(HG, HD).T], axis=1)
        # hidden^T packed: [s-chunk n][block j][s within chunk] contiguous
        # per partition: xTs[p, n, j, c] = hidden[b][512n+c, 128j+p]
        xT = hidden_state[b].T                       # [H, S]
        xTs = (xT.reshape(NHT, 128, NS, 512)
               .transpose(1, 2, 0, 3).reshape(128, NS * NHT * 512))
        in_maps.append({
            "xTs": np.ascontiguousarray(xTs.astype(bf)),
            "wqp": _pack_w(Wq[:, pcols].astype(bf)),
            "wkp": _pack_w(Wk[:, pcols].astype(bf)),
            "wvp": _pack_w(Wv[:, cols].astype(bf)),
            "bqk": np.ascontiguousarray(bqk_np),
            "bvb": np.ascontiguousarray(np.tile(bv[cols], (128, 1))),
            "cosT": cosT,
            "sinTs": sinTs,
            "swap": swap_m,
            "ones": ones_v,
            "wo": np.ascontiguousarray(Wo[cols, :].astype(bf)),
        })

    trace = bool(os.environ.get("KERNEL_TRACE"))
    res = bass_utils.run_bass_kernel_spmd(_NC, in_maps, core_ids=list(range(NCORES)),
                                          trace=trace)
    LAST_RESULTS = res

    out = np.zeros((B, S, H), np.float32)
    for c in range(NCORES):
        b = c // HG
        out[b] += res.results[c]["out"]
    out += bo[None, None, :]
    return out
